# revision 1
# baseline (speedup 1.0000x reference)
"""MoIE transformer block kernel.

Contract: kernel(**inputs) takes the FULL (unsharded) inputs and returns the
FULL [4, 2048, 1024] float32 output. Shapes are hardcoded per the problem
spec: B=4, S=2048, D=1024.

Computation (mirrors the reference exactly, fp32 throughout):
  ln1 = layernorm(x)
  for each of q,k,v:  comp/match/gate branch -> routed mix with ln1 passthrough
  attn = causal single-head attention (head_dim = D)
  o-branch on attn_out, residual add.

The work is organized row-sharded over 8 shards (batch b -> shards 2b, 2b+1),
each shard owning 1024 query rows; k/v for a batch are shared by its two
shards. This keeps the structure 1:1 with the 8-NeuronCore data-parallel
layout (2 cores per batch, causal-balanced query split).
"""

import math

import numpy as np

B, S, D = 4, 2048, 1024
EPS_LN = np.float32(1e-5)
EPS_MAS = np.float32(1e-9)
NEG_INF = np.float32(-1e9)


def _layernorm(x, g, b):
    m = np.mean(x, axis=-1, keepdims=True, dtype=np.float32)
    xc = x - m
    v = np.mean(np.square(xc), axis=-1, keepdims=True, dtype=np.float32)
    inv = np.float32(1.0) / np.sqrt(v + EPS_LN)
    return (xc * inv * g + b).astype(np.float32)


def _silu(x):
    return (x * (np.float32(1.0) / (np.float32(1.0) + np.exp(-x)))).astype(np.float32)


def _branch(x2d, proto_w, mu_w, mu_b, gate, passthrough):
    """x2d: [N, D] rows; returns routed output [N, D]."""
    scale = np.float32(1.0 / math.sqrt(D))
    match = (x2d @ proto_w.T) * scale
    comp = _silu(x2d @ mu_w.T + mu_b)
    cost = gate / (np.max(np.abs(gate)) + EPS_MAS)
    rl = match - cost
    mrl = np.maximum(rl, np.float32(0.0))
    mask = (rl > 0).astype(np.float32)
    return comp * mrl + passthrough * (np.float32(1.0) - mask)


def _causal_attention_rows(q_rows, k_full, v_full, q0):
    """q_rows: [M, D] queries at global offset q0; k/v: [S, D]. Causal."""
    scale = np.float32(1.0 / math.sqrt(D))
    scores = (q_rows @ k_full.T) * scale  # [M, S]
    m_idx = np.arange(q0, q0 + q_rows.shape[0])[:, None]
    k_idx = np.arange(k_full.shape[0])[None, :]
    scores = np.where(k_idx <= m_idx, scores, NEG_INF).astype(np.float32)
    smax = np.max(scores, axis=-1, keepdims=True)
    p = np.exp(scores - smax)
    p /= np.sum(p, axis=-1, keepdims=True, dtype=np.float32)
    return (p.astype(np.float32) @ v_full).astype(np.float32)


def kernel(x, ln_g, ln_b,
           q_mu_w, q_mu_b, q_proto, q_gate,
           k_mu_w, k_mu_b, k_proto, k_gate,
           v_mu_w, v_mu_b, v_proto, v_gate,
           o_mu_w, o_mu_b, o_proto, o_gate):
    x = np.asarray(x, dtype=np.float32)
    out = np.empty((B, S, D), dtype=np.float32)

    half = S // 2  # 1024 query rows per shard; 2 shards per batch

    for b in range(B):
        xb = x[b]  # [S, D]
        ln1 = _layernorm(xb, ln_g, ln_b)  # [S, D]

        # k/v branches over the full sequence (shared by both shards of b)
        kb = _branch(ln1, k_proto, k_mu_w, k_mu_b, k_gate, ln1)
        vb = _branch(ln1, v_proto, v_mu_w, v_mu_b, v_gate, ln1)

        for shard in range(2):
            r0, r1 = shard * half, (shard + 1) * half
            ln_rows = ln1[r0:r1]
            q_rows = _branch(ln_rows, q_proto, q_mu_w, q_mu_b, q_gate, ln_rows)
            attn = _causal_attention_rows(q_rows, kb, vb, r0)  # [half, D]
            o_rows = _branch(attn, o_proto, o_mu_w, o_mu_b, o_gate, attn)
            out[b, r0:r1] = xb[r0:r1] + o_rows

    return out



# revision 2
# speedup vs baseline: 1.0081x; 1.0081x over previous
"""MoIE transformer block — Trainium2 Bass/Tile kernel (8-core SPMD).

Sharding: core c -> batch b=c//2, parity par=c%2.  Each core owns the 8
row-tiles (128 rows each) of batch b with tile index == par (mod 2), i.e.
global tiles 2i+par.  Host permutes x row-tiles into OWN-FIRST order
(storage tiles 0..7 = own, 8..15 = partner) so the device program is
identical on every core; only input data differs.

Device layout is feature-on-partition ("transposed") almost everywhere:
  ln1T [d, r]  (bf16)   - built by PE-transposing row-major LN output
  kT   [d, kr] (bf16)   - k-branch in [o, r] orientation
  v    [kr, d] (bf16)   - v-branch in [o, r] then PE-transposed per block
  qT   [d, qr] (bf16)   - q-branch, own rows only
  scoresT [kr, qr] psum - attention computed transposed; softmax over the
                          partition (kr) axis: exp on ACT (scale=1/32 folded),
                          column sums via ones-matmul on PE, causal masking
                          via affine_select/memset/par-mask on the exp'd tiles
  attnT [d, qr] (bf16)  - pv matmul, normalized by 1/Z broadcast
  o-branch -> routed_oT [o, qr] fp32 -> PE transpose -> +x -> y

The routing chain (cost/threshold) runs per-partition (features on
partitions) via tensor_scalar with host-precomputed cost vectors; proto
weights are pre-scaled by 1/sqrt(D) on host.
"""
import sys

sys.path.insert(0, "/opt/trn_rl_repo")

import numpy as np

import concourse.bass as bass
import concourse.mybir as mybir
import concourse.tile as tile
from concourse.tile import ScopedClock

DT = mybir.dt
BF = DT.bfloat16
F32 = DT.float32
AF = mybir.ActivationFunctionType
ALU = mybir.AluOpType

NDT = 8          # d tiles (1024/128)
NOT = 8          # out-feature tiles
NRT = 16         # row tiles per batch
OWN = 8          # own row tiles per core
EPS_LN = 1e-5

_nop_counter = [0]


def _split_waits(nc):
    """This walrus build allows 1 sync-wait per instruction; split extras
    onto same-engine nops inserted just before."""
    f = nc.m.functions[0]
    for bb in f.blocks:
        insts = bb.instructions
        out = []
        changed = False
        for inst in insts:
            si = inst.sync_info
            waits = list(si.on_wait) if si is not None else []
            if len(waits) > 1:
                changed = True
                for w in waits[:-1]:
                    _nop_counter[0] += 1
                    nop = mybir.InstNoOp(name=f"wsplit-{_nop_counter[0]}")
                    nop.engine = inst.engine
                    nop.sync_info = mybir.SyncInfo(on_wait=[w], on_update=[])
                    out.append(nop)
                inst.sync_info = mybir.SyncInfo(
                    on_wait=[waits[-1]], on_update=list(si.on_update)
                )
            out.append(inst)
        if changed:
            bb.instructions = out


class TC(tile.TileContext):
    def _drain_and_barrier(self, tick_clock, wait_clock):
        nc = self.nc
        drain_inst = nc.sync.drain()
        wait_clock.add_sem_waits(
            drain_inst.ins, ScopedClock({None: tick_clock.global_clock})
        )
        nc.all_engine_barrier()
        assert self.sems is not None
        popped = self.nc._tile_sem_poison_stack.pop()
        assert popped is self._sem_poison
        self.nc.clear_and_free_semaphores(list(self.sems.allocated().values()))
        nc.all_engine_barrier()
        _split_waits(nc)


def install_ntff_hook():
    import types
    try:
        from antenv.axon_hooks import get_axon_ntff_profile_hook  # noqa
        return
    except ImportError:
        pass
    import antenv
    mod = types.ModuleType("antenv.axon_hooks")
    mod._hook = None
    def set_axon_ntff_profile_hook(h):
        mod._hook = h
    def get_axon_ntff_profile_hook():
        return mod._hook
    mod.set_axon_ntff_profile_hook = set_axon_ntff_profile_hook
    mod.get_axon_ntff_profile_hook = get_axon_ntff_profile_hook
    sys.modules["antenv.axon_hooks"] = mod
    antenv.axon_hooks = mod
    from trn_agent_boot.trn_boot import _ntff_profile_via_ctypes
    set_axon_ntff_profile_hook(
        _ntff_profile_via_ctypes("/opt/axon/libaxon_pjrt.so")
    )


def _chain(nc, work, out_sb, match_ps, z_ps, cost_ap, mub_ap, pass_ap,
           out_f32, offload=False):
    """Routing chain on a [128, 512] chunk, features on partitions.
    out = silu(z + mu_b) * relu(match - cost) + pass * (match - cost <= 0).
    The two SBUF-only tensor_tensor ops go to GpSimd (idle) when offload."""
    # rl = match - cost -> bf16 SBUF (sign-preserving; frees the PSUM bank
    # after a single read instead of three)
    rl = work.tile([128, 512], BF, tag="rl")
    nc.vector.tensor_scalar(
        out=rl, in0=match_ps, scalar1=cost_ap, scalar2=None,
        op0=ALU.subtract,
    )
    comp = work.tile([128, 512], BF, tag="comp")
    nc.scalar.activation(out=comp, in_=z_ps, func=AF.Silu, bias=mub_ap, scale=1.0)
    mrl = work.tile([128, 512], BF, tag="mrl")
    nc.scalar.activation(out=mrl, in_=rl, func=AF.Relu)
    m0 = work.tile([128, 512], BF, tag="m0")
    nc.vector.tensor_scalar(
        out=m0, in0=rl, scalar1=0.0, scalar2=None, op0=ALU.is_le,
    )
    t1 = work.tile([128, 512], BF, tag="t1")
    nc.vector.tensor_tensor(out=t1, in0=comp, in1=mrl, op=ALU.mult)
    t2 = work.tile([128, 512], F32 if out_f32 else BF, tag="t2")
    eng = nc.gpsimd if offload else nc.vector
    eng.tensor_tensor(out=t2, in0=pass_ap, in1=m0, op=ALU.mult)
    eng.tensor_tensor(out=out_sb, in0=t1, in1=t2, op=ALU.add)


def build_nc():
    nc = bass.Bass()

    x_in = nc.dram_tensor("x", [NRT, 128, 1024], F32, kind="ExternalInput")
    wt = {}
    for nm in ("q_proto", "q_mu", "k_proto", "k_mu",
               "v_proto", "v_mu", "o_proto", "o_mu"):
        wt[nm] = nc.dram_tensor(f"wt_{nm}", [NDT, 128, 1024], BF,
                                kind="ExternalInput")
    costT_in = nc.dram_tensor("costT", [128, 4, 8], F32, kind="ExternalInput")
    mubT_in = nc.dram_tensor("mubT", [128, 4, 8], F32, kind="ExternalInput")
    gbT_in = nc.dram_tensor("gbT", [128, 2, 8], F32, kind="ExternalInput")
    parm_in = nc.dram_tensor("par_mask", [128, 128], BF, kind="ExternalInput")
    y_out = nc.dram_tensor("y", [OWN, 128, 1024], F32, kind="ExternalOutput")

    BR = {"q": 0, "k": 1, "v": 2, "o": 3}  # costT/mubT branch index

    with TC(nc) as tc:
        from contextlib import ExitStack
        ctx = ExitStack()
        with ctx:
            singles = ctx.enter_context(tc.tile_pool(name="singles", bufs=1))
            wpool = ctx.enter_context(tc.tile_pool(name="wpool", bufs=2))
            work = ctx.enter_context(tc.tile_pool(name="work", bufs=2))
            # tags: mm0(2) + mm1(2) + acc(2) + misc(2) = 8 PSUM banks
            psum = ctx.enter_context(tc.tile_pool(name="psum", bufs=2, space="PSUM"))

            # ---- constants ----
            ident_bf = singles.tile([128, 128], BF)
            from concourse.masks import make_identity
            make_identity(nc, ident_bf)
            ident_f = singles.tile([128, 128], F32)
            make_identity(nc, ident_f)
            ones_bf = singles.tile([128, 128], BF)
            nc.vector.memset(ones_bf, 1.0)
            eps_t = singles.tile([128, 1], F32)
            nc.vector.memset(eps_t, EPS_LN)
            costT = singles.tile([128, 4, 8], F32)
            nc.sync.dma_start(out=costT, in_=costT_in[:, :, :])
            mubT = singles.tile([128, 4, 8], F32)
            nc.sync.dma_start(out=mubT, in_=mubT_in[:, :, :])
            gbT = singles.tile([128, 2, 8], F32)
            nc.sync.dma_start(out=gbT, in_=gbT_in[:, :, :])
            par_mask = singles.tile([128, 128], BF)
            nc.sync.dma_start(out=par_mask, in_=parm_in[:, :])

            # ---- persistent activations ----
            kT = singles.tile([128, NOT, 2048], BF)     # 4 MB
            v_sb = singles.tile([128, NRT, 1024], BF)   # 4 MB
            qT = singles.tile([128, NOT, 1024], BF)     # 2 MB
            attnT = singles.tile([128, NDT, 1024], BF)  # 2 MB

            # ---- weight loader ----
            def load_w(name):
                t = wpool.tile([128, NDT, 1024], BF, tag="w")
                nc.sync.dma_start(
                    out=t,
                    in_=wt[name].rearrange("j p o -> p j o"),
                )
                return t

            # =========== early scope: LN + branches ===========
            with tc.tile_pool(name="early", bufs=1) as early, \
                 tc.tile_pool(name="xearly", bufs=3) as xearly, \
                 tc.tile_pool(name="lnwork", bufs=3) as lnwork:

                ln1T = early.tile([128, NDT, 2048], BF)  # 4 MB

                # ---- Phase 1: LayerNorm + transpose ----
                for t in range(NRT):
                    xt = xearly.tile([128, 1024], F32, tag="x")
                    nc.sync.dma_start(out=xt, in_=x_in[t])
                    stats = lnwork.tile([128, 2, 6], F32, tag="stats")
                    nc.vector.bn_stats(out=stats[:, 0], in_=xt[:, 0:512])
                    nc.vector.bn_stats(out=stats[:, 1], in_=xt[:, 512:1024])
                    mv = lnwork.tile([128, 2], F32, tag="mv")
                    nc.vector.bn_aggr(out=mv, in_=stats)
                    rstd = lnwork.tile([128, 1], F32, tag="rstd")
                    nc.scalar.activation(
                        out=rstd, in_=mv[:, 1:2], func=AF.Sqrt,
                        bias=eps_t[:, 0:1], scale=1.0,
                    )
                    nc.vector.reciprocal(out=rstd, in_=rstd)
                    nmr = lnwork.tile([128, 1], F32, tag="nmr")
                    nc.vector.tensor_scalar(
                        out=nmr, in0=mv[:, 0:1], scalar1=rstd[:, 0:1],
                        scalar2=-1.0, op0=ALU.mult, op1=ALU.mult,
                    )
                    # normalize on ACT: (x * rstd) + (-m * rstd)
                    ln_rm = lnwork.tile([128, 1024], BF, tag="lnrm")
                    nc.scalar.activation(
                        out=ln_rm, in_=xt, func=AF.Identity,
                        bias=nmr[:, 0:1], scale=rstd[:, 0:1],
                    )
                    for j in range(NDT):
                        ps = psum.tile([128, 128], BF, tag="misc")
                        nc.tensor.transpose(
                            ps, ln_rm[:, j * 128:(j + 1) * 128], ident_bf
                        )
                        # fold ln_g / ln_b (per-partition in transposed layout)
                        nc.vector.tensor_scalar(
                            out=ln1T[:, j, t * 128:(t + 1) * 128],
                            in0=ps,
                            scalar1=gbT[:, 0, j:j + 1],
                            scalar2=gbT[:, 1, j:j + 1],
                            op0=ALU.mult, op1=ALU.add,
                        )

                # ---- Phase 2: k branch (o-on-partition, all 2048 rows) ----
                wkp = load_w("k_proto")
                wkm = load_w("k_mu")
                for r4 in range(4):
                    rsl = slice(r4 * 512, (r4 + 1) * 512)
                    for o8 in range(NOT):
                        osl = slice(o8 * 128, (o8 + 1) * 128)
                        mm = psum.tile([128, 512], F32, tag="mm0")
                        mz = psum.tile([128, 512], F32, tag="mm1")
                        for j in range(NDT):
                            nc.tensor.matmul(
                                mm, wkp[:, j, osl], ln1T[:, j, rsl],
                                start=(j == 0), stop=(j == NDT - 1),
                            )
                        for j in range(NDT):
                            nc.tensor.matmul(
                                mz, wkm[:, j, osl], ln1T[:, j, rsl],
                                start=(j == 0), stop=(j == NDT - 1),
                            )
                        _chain(nc, work, kT[:, o8, rsl], mm, mz,
                               costT[:, BR["k"], o8:o8 + 1],
                               mubT[:, BR["k"], o8:o8 + 1],
                               ln1T[:, o8, rsl], out_f32=False)

                # ---- Phase 3: v branch ([o, r] then transpose) ----
                wvp = load_w("v_proto")
                wvm = load_w("v_mu")
                for r4 in range(4):
                    rsl = slice(r4 * 512, (r4 + 1) * 512)
                    for o8 in range(NOT):
                        osl = slice(o8 * 128, (o8 + 1) * 128)
                        mm = psum.tile([128, 512], F32, tag="mm0")
                        mz = psum.tile([128, 512], F32, tag="mm1")
                        for j in range(NDT):
                            nc.tensor.matmul(
                                mm, wvp[:, j, osl], ln1T[:, j, rsl],
                                start=(j == 0), stop=(j == NDT - 1),
                            )
                        for j in range(NDT):
                            nc.tensor.matmul(
                                mz, wvm[:, j, osl], ln1T[:, j, rsl],
                                start=(j == 0), stop=(j == NDT - 1),
                            )
                        vt = work.tile([128, 512], BF, tag="vt")
                        _chain(nc, work, vt, mm, mz,
                               costT[:, BR["v"], o8:o8 + 1],
                               mubT[:, BR["v"], o8:o8 + 1],
                               ln1T[:, o8, rsl], out_f32=False)
                        for u in range(4):
                            ps = psum.tile([128, 128], BF, tag="misc")
                            nc.tensor.transpose(
                                ps, vt[:, u * 128:(u + 1) * 128], ident_bf
                            )
                            nc.vector.tensor_copy(
                                out=v_sb[:, r4 * 4 + u, osl], in_=ps
                            )

                # ---- Phase 4: q branch (own rows only) ----
                wqp = load_w("q_proto")
                wqm = load_w("q_mu")
                for g in range(2):
                    gsl = slice(g * 512, (g + 1) * 512)
                    for o8 in range(NOT):
                        osl = slice(o8 * 128, (o8 + 1) * 128)
                        mm = psum.tile([128, 512], F32, tag="mm0")
                        mz = psum.tile([128, 512], F32, tag="mm1")
                        for j in range(NDT):
                            nc.tensor.matmul(
                                mm, wqp[:, j, osl], ln1T[:, j, gsl],
                                start=(j == 0), stop=(j == NDT - 1),
                            )
                        for j in range(NDT):
                            nc.tensor.matmul(
                                mz, wqm[:, j, osl], ln1T[:, j, gsl],
                                start=(j == 0), stop=(j == NDT - 1),
                            )
                        _chain(nc, work, qT[:, o8, gsl], mm, mz,
                               costT[:, BR["q"], o8:o8 + 1],
                               mubT[:, BR["q"], o8:o8 + 1],
                               ln1T[:, o8, gsl], out_f32=False)

            # =========== late scope: attention + o + output ===========
            wop = load_w("o_proto")
            wom = load_w("o_mu")
            with tc.tile_pool(name="late", bufs=1) as late, \
                 tc.tile_pool(name="xlate", bufs=1) as xlate, \
                 tc.tile_pool(name="lwork", bufs=1) as lwork:

                pT_g = [late.tile([128, 8, 512], BF, name="pT0"),
                        late.tile([128, 16, 512], BF, name="pT1")]

                for g in range(2):
                    gsl = slice(g * 512, (g + 1) * 512)
                    nk = 4 * (g + 1)
                    klist = list(range(nk)) + list(range(8, 8 + nk))
                    pT = pT_g[g]

                    # scoresT + exp per storage krtile
                    for i, k in enumerate(klist):
                        ksl = slice(k * 128, (k + 1) * 128)
                        sc = psum.tile([128, 512], F32, tag="mm0")
                        for j in range(NDT):
                            nc.tensor.matmul(
                                sc, kT[:, j, ksl], qT[:, j, gsl],
                                start=(j == 0), stop=(j == NDT - 1),
                            )
                        nc.scalar.activation(
                            out=pT[:, i, :], in_=sc, func=AF.Exp,
                            bias=0.0, scale=0.03125,
                        )

                    # causal masks on exp'd tiles
                    for u0 in range(4):
                        # own diagonal tile k = 4g+u0 (dense idx = k)
                        i_own = 4 * g + u0
                        blk = pT[:, i_own, u0 * 128:(u0 + 1) * 128]
                        # keep where kr_local(p) <= q_local(jj):  jj - p >= 0
                        nc.gpsimd.affine_select(
                            out=blk, in_=blk,
                            pattern=[[1, 128]], compare_op=ALU.is_ge,
                            fill=0.0, base=0, channel_multiplier=-1,
                        )
                        if u0 > 0:
                            nc.gpsimd.memset(pT[:, i_own, 0:u0 * 128], 0.0)
                        # partner tile k' = 4g+u0 (dense idx = nk + 4g+u0)
                        i_par = nk + 4 * g + u0
                        pblk = pT[:, i_par, u0 * 128:(u0 + 1) * 128]
                        nc.vector.tensor_tensor(
                            out=pblk, in0=pblk, in1=par_mask, op=ALU.mult,
                        )
                        if u0 > 0:
                            nc.gpsimd.memset(pT[:, i_par, 0:u0 * 128], 0.0)

                    # Z = column sums via ones-outer-product matmul; every
                    # output row holds the column sum -> broadcast for free.
                    zps = psum.tile([128, 512], F32, tag="misc")
                    for i in range(len(klist)):
                        nc.tensor.matmul(
                            zps, ones_bf, pT[:, i, :],
                            start=(i == 0), stop=(i == len(klist) - 1),
                        )
                    zb = lwork.tile([128, 512], F32, tag="zb")
                    nc.vector.reciprocal(out=zb, in_=zps)

                    # attnT = (v.T @ pT) * zb
                    for j in range(NDT):
                        jsl = slice(j * 128, (j + 1) * 128)
                        ap = psum.tile([128, 512], F32, tag="acc")
                        for i, k in enumerate(klist):
                            nc.tensor.matmul(
                                ap, v_sb[:, k, jsl], pT[:, i, :],
                                start=(i == 0), stop=(i == len(klist) - 1),
                            )
                        nc.vector.tensor_tensor(
                            out=attnT[:, j, gsl], in0=ap, in1=zb, op=ALU.mult,
                        )

                    # o branch + transpose + residual + store
                    xr = []
                    for u in range(4):
                        xt = xlate.tile([128, 1024], F32, tag=f"xr{u}",
                                        name=f"xr{g}_{u}")
                        nc.sync.dma_start(out=xt, in_=x_in[4 * g + u])
                        xr.append(xt)
                    for o8 in range(NOT):
                        osl = slice(o8 * 128, (o8 + 1) * 128)
                        mm = psum.tile([128, 512], F32, tag="mm0")
                        mz = psum.tile([128, 512], F32, tag="mm1")
                        for j in range(NDT):
                            nc.tensor.matmul(
                                mm, wop[:, j, osl], attnT[:, j, gsl],
                                start=(j == 0), stop=(j == NDT - 1),
                            )
                        for j in range(NDT):
                            nc.tensor.matmul(
                                mz, wom[:, j, osl], attnT[:, j, gsl],
                                start=(j == 0), stop=(j == NDT - 1),
                            )
                        rout = work.tile([128, 512], BF, tag="rout")
                        _chain(nc, work, rout, mm, mz,
                               costT[:, BR["o"], o8:o8 + 1],
                               mubT[:, BR["o"], o8:o8 + 1],
                               attnT[:, o8, gsl], out_f32=False)
                        for u in range(4):
                            ps = psum.tile([128, 128], BF, tag="misc")
                            nc.tensor.transpose(
                                ps, rout[:, u * 128:(u + 1) * 128], ident_bf
                            )
                            op = work.tile([128, 128], F32, tag="opiece")
                            nc.vector.tensor_tensor(
                                out=op, in0=ps, in1=xr[u][:, osl], op=ALU.add,
                            )
                            nc.sync.dma_start(
                                out=y_out[4 * g + u][:, osl], in_=op
                            )

    return nc


# ---------------- host side ----------------

def make_in_maps(inputs):
    import ml_dtypes
    bf16 = ml_dtypes.bfloat16
    x = np.asarray(inputs["x"], np.float32)
    scale = np.float32(1.0 / 32.0)

    wts = {}
    for nm, proto_key, mu_key in (
        ("q", "q_proto", "q_mu_w"), ("k", "k_proto", "k_mu_w"),
        ("v", "v_proto", "v_mu_w"), ("o", "o_proto", "o_mu_w"),
    ):
        p = np.asarray(inputs[proto_key], np.float32) * scale
        m = np.asarray(inputs[mu_key], np.float32)
        wts[f"wt_{nm}_proto"] = np.ascontiguousarray(
            p.T.reshape(NDT, 128, 1024).astype(bf16))
        wts[f"wt_{nm}_mu"] = np.ascontiguousarray(
            m.T.reshape(NDT, 128, 1024).astype(bf16))

    costT = np.zeros((128, 4, 8), np.float32)
    mubT = np.zeros((128, 4, 8), np.float32)
    for bi, nm in enumerate(("q", "k", "v", "o")):
        gate = np.asarray(inputs[f"{nm}_gate"], np.float32)
        cost = gate / (np.max(np.abs(gate)) + np.float32(1e-9))
        costT[:, bi, :] = cost.reshape(8, 128).T
        mubT[:, bi, :] = np.asarray(
            inputs[f"{nm}_mu_b"], np.float32).reshape(8, 128).T

    gbT = np.zeros((128, 2, 8), np.float32)
    gbT[:, 0, :] = np.asarray(inputs["ln_g"], np.float32).reshape(8, 128).T
    gbT[:, 1, :] = np.asarray(inputs["ln_b"], np.float32).reshape(8, 128).T

    in_maps = []
    for c in range(8):
        b, par = c // 2, c % 2
        xt = x[b].reshape(16, 128, 1024)
        perm = [2 * i + par for i in range(8)] + \
               [2 * i + (1 - par) for i in range(8)]
        m = {
            "x": np.ascontiguousarray(xt[perm]),
            "costT": costT, "mubT": mubT, "gbT": gbT,
            "par_mask": np.full((128, 128), par, bf16),
        }
        m.update(wts)
        in_maps.append(m)
    return in_maps


def assemble_output(results):
    out = np.empty((4, 2048, 1024), np.float32)
    for c in range(8):
        b, par = c // 2, c % 2
        y = results[c]["y"]  # [8, 128, 1024]
        ob = out[b].reshape(16, 128, 1024)
        for i in range(8):
            ob[2 * i + par] = y[i]
    return out


_CACHE = {}


def run(inputs, trace=False, tmpdir=None):
    install_ntff_hook()
    from concourse.bass_utils import run_bass_kernel_spmd
    if "nc" not in _CACHE:
        _CACHE["nc"] = build_nc()
    nc = _CACHE["nc"]
    in_maps = make_in_maps(inputs)
    r = run_bass_kernel_spmd(
        nc, in_maps, core_ids=list(range(8)), trace=trace, tmpdir=tmpdir,
    )
    return assemble_output(r.results), r


def kernel(**inputs):
    out, _ = run(inputs, trace=False)
    return out


# revision 3
# speedup vs baseline: 1.0189x; 1.0108x over previous
"""MoIE transformer block — Trainium2 Bass/Tile kernel (8-core SPMD).

Sharding: core c -> batch b=c//2, parity par=c%2.  Each core owns the 8
row-tiles (128 rows each) of batch b with tile index == par (mod 2), i.e.
global tiles 2i+par.  Host permutes x row-tiles into OWN-FIRST order
(storage tiles 0..7 = own, 8..15 = partner) so the device program is
identical on every core; only input data differs.

Device layout is feature-on-partition ("transposed") almost everywhere:
  ln1T [d, r]  (bf16)   - built by PE-transposing row-major LN output
  kT   [d, kr] (bf16)   - k-branch in [o, r] orientation
  v    [kr, d] (bf16)   - v-branch in [o, r] then PE-transposed per block
  qT   [d, qr] (bf16)   - q-branch, own rows only
  scoresT [kr, qr] psum - attention computed transposed; softmax over the
                          partition (kr) axis: exp on ACT (scale=1/32 folded),
                          column sums via ones-matmul on PE, causal masking
                          via affine_select/memset/par-mask on the exp'd tiles
  attnT [d, qr] (bf16)  - pv matmul, normalized by 1/Z broadcast
  o-branch -> routed_oT [o, qr] fp32 -> PE transpose -> +x -> y

The routing chain (cost/threshold) runs per-partition (features on
partitions) via tensor_scalar with host-precomputed cost vectors; proto
weights are pre-scaled by 1/sqrt(D) on host.
"""
import sys

sys.path.insert(0, "/opt/trn_rl_repo")

import numpy as np

import concourse.bass as bass
import concourse.mybir as mybir
import concourse.tile as tile
from concourse.tile import ScopedClock

DT = mybir.dt
BF = DT.bfloat16
F32 = DT.float32
AF = mybir.ActivationFunctionType
ALU = mybir.AluOpType

NDT = 8          # d tiles (1024/128)
NOT = 8          # out-feature tiles
NRT = 16         # row tiles per batch
OWN = 8          # own row tiles per core
EPS_LN = 1e-5

_nop_counter = [0]


def _split_waits(nc):
    """This walrus build allows 1 sync-wait per instruction; split extras
    onto same-engine nops inserted just before."""
    f = nc.m.functions[0]
    for bb in f.blocks:
        insts = bb.instructions
        out = []
        changed = False
        for inst in insts:
            si = inst.sync_info
            waits = list(si.on_wait) if si is not None else []
            if len(waits) > 1:
                changed = True
                for w in waits[:-1]:
                    _nop_counter[0] += 1
                    nop = mybir.InstNoOp(name=f"wsplit-{_nop_counter[0]}")
                    nop.engine = inst.engine
                    nop.sync_info = mybir.SyncInfo(on_wait=[w], on_update=[])
                    out.append(nop)
                inst.sync_info = mybir.SyncInfo(
                    on_wait=[waits[-1]], on_update=list(si.on_update)
                )
            out.append(inst)
        if changed:
            bb.instructions = out


class TC(tile.TileContext):
    def _drain_and_barrier(self, tick_clock, wait_clock):
        nc = self.nc
        drain_inst = nc.sync.drain()
        wait_clock.add_sem_waits(
            drain_inst.ins, ScopedClock({None: tick_clock.global_clock})
        )
        nc.all_engine_barrier()
        assert self.sems is not None
        popped = self.nc._tile_sem_poison_stack.pop()
        assert popped is self._sem_poison
        self.nc.clear_and_free_semaphores(list(self.sems.allocated().values()))
        nc.all_engine_barrier()
        _split_waits(nc)


def install_ntff_hook():
    """Best-effort: register the axon NTFF profile hook the slim container
    lacks.  Only needed for trace=True; failures are ignored."""
    import types
    try:
        from antenv.axon_hooks import get_axon_ntff_profile_hook  # noqa
        return
    except ImportError:
        pass
    try:
        import antenv
        mod = types.ModuleType("antenv.axon_hooks")
        mod._hook = None
        def set_axon_ntff_profile_hook(h):
            mod._hook = h
        def get_axon_ntff_profile_hook():
            return mod._hook
        mod.set_axon_ntff_profile_hook = set_axon_ntff_profile_hook
        mod.get_axon_ntff_profile_hook = get_axon_ntff_profile_hook
        sys.modules["antenv.axon_hooks"] = mod
        antenv.axon_hooks = mod
        from trn_agent_boot.trn_boot import _ntff_profile_via_ctypes
        set_axon_ntff_profile_hook(
            _ntff_profile_via_ctypes("/opt/axon/libaxon_pjrt.so")
        )
    except Exception:
        pass


def _enable_jax_cache():
    try:
        import jax
        jax.config.update("jax_compilation_cache_dir", "/tmp/jax_moie_cache")
        jax.config.update("jax_persistent_cache_min_entry_size_bytes", 0)
        jax.config.update("jax_persistent_cache_min_compile_time_secs", 0.0)
    except Exception:
        pass


def _chain(nc, work, out_sb, match_ps, z_ps, cost_ap, mub_ap, pass_ap,
           out_f32, offload=False):
    """Routing chain on a [128, 512] chunk, features on partitions.
    out = silu(z + mu_b) * relu(match - cost) + pass * (match - cost <= 0).
    The two SBUF-only tensor_tensor ops go to GpSimd (idle) when offload."""
    # rl = match - cost -> bf16 SBUF (sign-preserving; frees the PSUM bank
    # after a single read instead of three)
    rl = work.tile([128, 512], BF, tag="rl")
    nc.vector.tensor_scalar(
        out=rl, in0=match_ps, scalar1=cost_ap, scalar2=None,
        op0=ALU.subtract,
    )
    comp = work.tile([128, 512], BF, tag="comp")
    nc.scalar.activation(out=comp, in_=z_ps, func=AF.Silu, bias=mub_ap, scale=1.0)
    mrl = work.tile([128, 512], BF, tag="mrl")
    nc.scalar.activation(out=mrl, in_=rl, func=AF.Relu)
    m0 = work.tile([128, 512], BF, tag="m0")
    nc.vector.tensor_scalar(
        out=m0, in0=rl, scalar1=0.0, scalar2=None, op0=ALU.is_le,
    )
    t1 = work.tile([128, 512], BF, tag="t1")
    nc.vector.tensor_tensor(out=t1, in0=comp, in1=mrl, op=ALU.mult)
    t2 = work.tile([128, 512], F32 if out_f32 else BF, tag="t2")
    eng = nc.gpsimd if offload else nc.vector
    eng.tensor_tensor(out=t2, in0=pass_ap, in1=m0, op=ALU.mult)
    eng.tensor_tensor(out=out_sb, in0=t1, in1=t2, op=ALU.add)


def build_nc():
    nc = bass.Bass()

    x_in = nc.dram_tensor("x", [NRT, 128, 1024], F32, kind="ExternalInput")
    wt = {}
    for nm in ("q_proto", "q_mu", "k_proto", "k_mu",
               "v_proto", "v_mu", "o_proto", "o_mu"):
        wt[nm] = nc.dram_tensor(f"wt_{nm}", [NDT, 128, 1024], BF,
                                kind="ExternalInput")
    costT_in = nc.dram_tensor("costT", [128, 4, 8], F32, kind="ExternalInput")
    mubT_in = nc.dram_tensor("mubT", [128, 4, 8], F32, kind="ExternalInput")
    gbT_in = nc.dram_tensor("gbT", [128, 2, 8], F32, kind="ExternalInput")
    parm_in = nc.dram_tensor("par_mask", [128, 128], BF, kind="ExternalInput")
    y_out = nc.dram_tensor("y", [OWN, 128, 1024], F32, kind="ExternalOutput")

    BR = {"q": 0, "k": 1, "v": 2, "o": 3}  # costT/mubT branch index

    with TC(nc) as tc:
        from contextlib import ExitStack
        ctx = ExitStack()
        with ctx:
            singles = ctx.enter_context(tc.tile_pool(name="singles", bufs=1))
            wpool = ctx.enter_context(tc.tile_pool(name="wpool", bufs=2))
            work = ctx.enter_context(tc.tile_pool(name="work", bufs=2))
            # tags: mm0(2) + mm1(2) + acc(2) + misc(2) = 8 PSUM banks
            psum = ctx.enter_context(tc.tile_pool(name="psum", bufs=2, space="PSUM"))

            # ---- constants ----
            ident_bf = singles.tile([128, 128], BF)
            from concourse.masks import make_identity
            make_identity(nc, ident_bf)
            ident_f = singles.tile([128, 128], F32)
            make_identity(nc, ident_f)
            ones_bf = singles.tile([128, 128], BF)
            nc.vector.memset(ones_bf, 1.0)
            eps_t = singles.tile([128, 1], F32)
            nc.vector.memset(eps_t, EPS_LN)
            costT = singles.tile([128, 4, 8], F32)
            nc.sync.dma_start(out=costT, in_=costT_in[:, :, :])
            mubT = singles.tile([128, 4, 8], F32)
            nc.sync.dma_start(out=mubT, in_=mubT_in[:, :, :])
            gbT = singles.tile([128, 2, 8], F32)
            nc.sync.dma_start(out=gbT, in_=gbT_in[:, :, :])
            par_mask = singles.tile([128, 128], BF)
            nc.sync.dma_start(out=par_mask, in_=parm_in[:, :])

            # ---- persistent activations ----
            kT = singles.tile([128, NOT, 2048], BF)     # 4 MB
            v_sb = singles.tile([128, NRT, 1024], BF)   # 4 MB
            qT = singles.tile([128, NOT, 1024], BF)     # 2 MB
            attnT = singles.tile([128, NDT, 1024], BF)  # 2 MB

            # ---- weight loader ----
            def load_w(name):
                t = wpool.tile([128, NDT, 1024], BF, tag="w")
                nc.sync.dma_start(
                    out=t,
                    in_=wt[name].rearrange("j p o -> p j o"),
                )
                return t

            # =========== early scope: LN + branches ===========
            with tc.tile_pool(name="early", bufs=1) as early, \
                 tc.tile_pool(name="xearly", bufs=3) as xearly, \
                 tc.tile_pool(name="lnwork", bufs=3) as lnwork:

                ln1T = early.tile([128, NDT, 2048], BF)  # 4 MB

                # ---- Phase 1: LayerNorm + transpose ----
                for t in range(NRT):
                    xt = xearly.tile([128, 1024], F32, tag="x")
                    nc.sync.dma_start(out=xt, in_=x_in[t])
                    stats = lnwork.tile([128, 2, 6], F32, tag="stats")
                    nc.vector.bn_stats(out=stats[:, 0], in_=xt[:, 0:512])
                    nc.vector.bn_stats(out=stats[:, 1], in_=xt[:, 512:1024])
                    mv = lnwork.tile([128, 2], F32, tag="mv")
                    nc.vector.bn_aggr(out=mv, in_=stats)
                    rstd = lnwork.tile([128, 1], F32, tag="rstd")
                    nc.scalar.activation(
                        out=rstd, in_=mv[:, 1:2], func=AF.Sqrt,
                        bias=eps_t[:, 0:1], scale=1.0,
                    )
                    nc.vector.reciprocal(out=rstd, in_=rstd)
                    nmr = lnwork.tile([128, 1], F32, tag="nmr")
                    nc.vector.tensor_scalar(
                        out=nmr, in0=mv[:, 0:1], scalar1=rstd[:, 0:1],
                        scalar2=-1.0, op0=ALU.mult, op1=ALU.mult,
                    )
                    # normalize on ACT: (x * rstd) + (-m * rstd)
                    ln_rm = lnwork.tile([128, 1024], BF, tag="lnrm")
                    nc.scalar.activation(
                        out=ln_rm, in_=xt, func=AF.Identity,
                        bias=nmr[:, 0:1], scale=rstd[:, 0:1],
                    )
                    for j in range(NDT):
                        ps = psum.tile([128, 128], BF, tag="misc")
                        nc.tensor.transpose(
                            ps, ln_rm[:, j * 128:(j + 1) * 128], ident_bf
                        )
                        # fold ln_g / ln_b (per-partition in transposed
                        # layout); alternate DVE/ACT to halve the serial
                        # producer latency
                        if j % 2 == 0:
                            nc.vector.tensor_scalar(
                                out=ln1T[:, j, t * 128:(t + 1) * 128],
                                in0=ps,
                                scalar1=gbT[:, 0, j:j + 1],
                                scalar2=gbT[:, 1, j:j + 1],
                                op0=ALU.mult, op1=ALU.add,
                            )
                        else:
                            nc.scalar.activation(
                                out=ln1T[:, j, t * 128:(t + 1) * 128],
                                in_=ps, func=AF.Identity,
                                bias=gbT[:, 1, j:j + 1],
                                scale=gbT[:, 0, j:j + 1],
                            )

                # ---- Phase 2: k branch (o-on-partition, all 2048 rows) ----
                wkp = load_w("k_proto")
                wkm = load_w("k_mu")
                for r4 in range(4):
                    rsl = slice(r4 * 512, (r4 + 1) * 512)
                    for o8 in range(NOT):
                        osl = slice(o8 * 128, (o8 + 1) * 128)
                        mm = psum.tile([128, 512], F32, tag="mm0")
                        mz = psum.tile([128, 512], F32, tag="mm1")
                        for j in range(NDT):
                            nc.tensor.matmul(
                                mm, wkp[:, j, osl], ln1T[:, j, rsl],
                                start=(j == 0), stop=(j == NDT - 1),
                            )
                        for j in range(NDT):
                            nc.tensor.matmul(
                                mz, wkm[:, j, osl], ln1T[:, j, rsl],
                                start=(j == 0), stop=(j == NDT - 1),
                            )
                        _chain(nc, work, kT[:, o8, rsl], mm, mz,
                               costT[:, BR["k"], o8:o8 + 1],
                               mubT[:, BR["k"], o8:o8 + 1],
                               ln1T[:, o8, rsl], out_f32=False)

                # ---- Phase 3: v branch ([o, r] then transpose) ----
                wvp = load_w("v_proto")
                wvm = load_w("v_mu")
                for r4 in range(4):
                    rsl = slice(r4 * 512, (r4 + 1) * 512)
                    for o8 in range(NOT):
                        osl = slice(o8 * 128, (o8 + 1) * 128)
                        mm = psum.tile([128, 512], F32, tag="mm0")
                        mz = psum.tile([128, 512], F32, tag="mm1")
                        for j in range(NDT):
                            nc.tensor.matmul(
                                mm, wvp[:, j, osl], ln1T[:, j, rsl],
                                start=(j == 0), stop=(j == NDT - 1),
                            )
                        for j in range(NDT):
                            nc.tensor.matmul(
                                mz, wvm[:, j, osl], ln1T[:, j, rsl],
                                start=(j == 0), stop=(j == NDT - 1),
                            )
                        vt = work.tile([128, 512], BF, tag="vt")
                        _chain(nc, work, vt, mm, mz,
                               costT[:, BR["v"], o8:o8 + 1],
                               mubT[:, BR["v"], o8:o8 + 1],
                               ln1T[:, o8, rsl], out_f32=False)
                        for u in range(4):
                            ps = psum.tile([128, 128], BF, tag="misc")
                            nc.tensor.transpose(
                                ps, vt[:, u * 128:(u + 1) * 128], ident_bf
                            )
                            nc.vector.tensor_copy(
                                out=v_sb[:, r4 * 4 + u, osl], in_=ps
                            )

                # ---- Phase 4: q branch (own rows only) ----
                wqp = load_w("q_proto")
                wqm = load_w("q_mu")
                for g in range(2):
                    gsl = slice(g * 512, (g + 1) * 512)
                    for o8 in range(NOT):
                        osl = slice(o8 * 128, (o8 + 1) * 128)
                        mm = psum.tile([128, 512], F32, tag="mm0")
                        mz = psum.tile([128, 512], F32, tag="mm1")
                        for j in range(NDT):
                            nc.tensor.matmul(
                                mm, wqp[:, j, osl], ln1T[:, j, gsl],
                                start=(j == 0), stop=(j == NDT - 1),
                            )
                        for j in range(NDT):
                            nc.tensor.matmul(
                                mz, wqm[:, j, osl], ln1T[:, j, gsl],
                                start=(j == 0), stop=(j == NDT - 1),
                            )
                        _chain(nc, work, qT[:, o8, gsl], mm, mz,
                               costT[:, BR["q"], o8:o8 + 1],
                               mubT[:, BR["q"], o8:o8 + 1],
                               ln1T[:, o8, gsl], out_f32=False)

            # =========== late scope: attention + o + output ===========
            wop = load_w("o_proto")
            wom = load_w("o_mu")
            with tc.tile_pool(name="late", bufs=1) as late, \
                 tc.tile_pool(name="xlate", bufs=1) as xlate, \
                 tc.tile_pool(name="lwork", bufs=1) as lwork:

                pT_g = [late.tile([128, 8, 512], BF, name="pT0"),
                        late.tile([128, 16, 512], BF, name="pT1")]

                for g in range(2):
                    gsl = slice(g * 512, (g + 1) * 512)
                    nk = 4 * (g + 1)
                    klist = list(range(nk)) + list(range(8, 8 + nk))
                    pT = pT_g[g]

                    # scoresT + exp per storage krtile
                    for i, k in enumerate(klist):
                        ksl = slice(k * 128, (k + 1) * 128)
                        sc = psum.tile([128, 512], F32, tag="mm0")
                        for j in range(NDT):
                            nc.tensor.matmul(
                                sc, kT[:, j, ksl], qT[:, j, gsl],
                                start=(j == 0), stop=(j == NDT - 1),
                            )
                        nc.scalar.activation(
                            out=pT[:, i, :], in_=sc, func=AF.Exp,
                            bias=0.0, scale=0.03125,
                        )

                    # causal masks on exp'd tiles
                    for u0 in range(4):
                        # own diagonal tile k = 4g+u0 (dense idx = k)
                        i_own = 4 * g + u0
                        blk = pT[:, i_own, u0 * 128:(u0 + 1) * 128]
                        # keep where kr_local(p) <= q_local(jj):  jj - p >= 0
                        nc.gpsimd.affine_select(
                            out=blk, in_=blk,
                            pattern=[[1, 128]], compare_op=ALU.is_ge,
                            fill=0.0, base=0, channel_multiplier=-1,
                        )
                        if u0 > 0:
                            nc.gpsimd.memset(pT[:, i_own, 0:u0 * 128], 0.0)
                        # partner tile k' = 4g+u0 (dense idx = nk + 4g+u0)
                        i_par = nk + 4 * g + u0
                        pblk = pT[:, i_par, u0 * 128:(u0 + 1) * 128]
                        nc.vector.tensor_tensor(
                            out=pblk, in0=pblk, in1=par_mask, op=ALU.mult,
                        )
                        if u0 > 0:
                            nc.gpsimd.memset(pT[:, i_par, 0:u0 * 128], 0.0)

                    # Z = column sums via ones-outer-product matmul; every
                    # output row holds the column sum -> broadcast for free.
                    zps = psum.tile([128, 512], F32, tag="misc")
                    for i in range(len(klist)):
                        nc.tensor.matmul(
                            zps, ones_bf, pT[:, i, :],
                            start=(i == 0), stop=(i == len(klist) - 1),
                        )
                    zb = lwork.tile([128, 512], F32, tag="zb")
                    nc.vector.reciprocal(out=zb, in_=zps)

                    # attnT = (v.T @ pT) * zb
                    for j in range(NDT):
                        jsl = slice(j * 128, (j + 1) * 128)
                        ap = psum.tile([128, 512], F32, tag="acc")
                        for i, k in enumerate(klist):
                            nc.tensor.matmul(
                                ap, v_sb[:, k, jsl], pT[:, i, :],
                                start=(i == 0), stop=(i == len(klist) - 1),
                            )
                        nc.vector.tensor_tensor(
                            out=attnT[:, j, gsl], in0=ap, in1=zb, op=ALU.mult,
                        )

                    # o branch + transpose + residual + store
                    xr = []
                    for u in range(4):
                        xt = xlate.tile([128, 1024], F32, tag=f"xr{u}",
                                        name=f"xr{g}_{u}")
                        nc.sync.dma_start(out=xt, in_=x_in[4 * g + u])
                        xr.append(xt)
                    for o8 in range(NOT):
                        osl = slice(o8 * 128, (o8 + 1) * 128)
                        mm = psum.tile([128, 512], F32, tag="mm0")
                        mz = psum.tile([128, 512], F32, tag="mm1")
                        for j in range(NDT):
                            nc.tensor.matmul(
                                mm, wop[:, j, osl], attnT[:, j, gsl],
                                start=(j == 0), stop=(j == NDT - 1),
                            )
                        for j in range(NDT):
                            nc.tensor.matmul(
                                mz, wom[:, j, osl], attnT[:, j, gsl],
                                start=(j == 0), stop=(j == NDT - 1),
                            )
                        rout = work.tile([128, 512], BF, tag="rout")
                        _chain(nc, work, rout, mm, mz,
                               costT[:, BR["o"], o8:o8 + 1],
                               mubT[:, BR["o"], o8:o8 + 1],
                               attnT[:, o8, gsl], out_f32=False)
                        for u in range(4):
                            ps = psum.tile([128, 128], BF, tag="misc")
                            nc.tensor.transpose(
                                ps, rout[:, u * 128:(u + 1) * 128], ident_bf
                            )
                            op = work.tile([128, 128], F32, tag="opiece")
                            nc.vector.tensor_tensor(
                                out=op, in0=ps, in1=xr[u][:, osl], op=ALU.add,
                            )
                            nc.sync.dma_start(
                                out=y_out[4 * g + u][:, osl], in_=op
                            )

    return nc


# ---------------- host side ----------------

def make_in_maps(inputs):
    import ml_dtypes
    bf16 = ml_dtypes.bfloat16
    x = np.asarray(inputs["x"], np.float32)
    scale = np.float32(1.0 / 32.0)

    wts = {}
    for nm, proto_key, mu_key in (
        ("q", "q_proto", "q_mu_w"), ("k", "k_proto", "k_mu_w"),
        ("v", "v_proto", "v_mu_w"), ("o", "o_proto", "o_mu_w"),
    ):
        p = np.asarray(inputs[proto_key], np.float32) * scale
        m = np.asarray(inputs[mu_key], np.float32)
        wts[f"wt_{nm}_proto"] = np.ascontiguousarray(
            p.T.reshape(NDT, 128, 1024).astype(bf16))
        wts[f"wt_{nm}_mu"] = np.ascontiguousarray(
            m.T.reshape(NDT, 128, 1024).astype(bf16))

    costT = np.zeros((128, 4, 8), np.float32)
    mubT = np.zeros((128, 4, 8), np.float32)
    for bi, nm in enumerate(("q", "k", "v", "o")):
        gate = np.asarray(inputs[f"{nm}_gate"], np.float32)
        cost = gate / (np.max(np.abs(gate)) + np.float32(1e-9))
        costT[:, bi, :] = cost.reshape(8, 128).T
        mubT[:, bi, :] = np.asarray(
            inputs[f"{nm}_mu_b"], np.float32).reshape(8, 128).T

    gbT = np.zeros((128, 2, 8), np.float32)
    gbT[:, 0, :] = np.asarray(inputs["ln_g"], np.float32).reshape(8, 128).T
    gbT[:, 1, :] = np.asarray(inputs["ln_b"], np.float32).reshape(8, 128).T

    in_maps = []
    for c in range(8):
        b, par = c // 2, c % 2
        xt = x[b].reshape(16, 128, 1024)
        perm = [2 * i + par for i in range(8)] + \
               [2 * i + (1 - par) for i in range(8)]
        m = {
            "x": np.ascontiguousarray(xt[perm]),
            "costT": costT, "mubT": mubT, "gbT": gbT,
            "par_mask": np.full((128, 128), par, bf16),
        }
        m.update(wts)
        in_maps.append(m)
    return in_maps


def assemble_output(results):
    out = np.empty((4, 2048, 1024), np.float32)
    for c in range(8):
        b, par = c // 2, c % 2
        y = results[c]["y"]  # [8, 128, 1024]
        ob = out[b].reshape(16, 128, 1024)
        for i in range(8):
            ob[2 * i + par] = y[i]
    return out


_CACHE = {}


def run(inputs, trace=False, tmpdir=None):
    install_ntff_hook()
    from concourse.bass_utils import run_bass_kernel_spmd
    if "nc" not in _CACHE:
        _CACHE["nc"] = build_nc()
    nc = _CACHE["nc"]
    in_maps = make_in_maps(inputs)
    r = run_bass_kernel_spmd(
        nc, in_maps, core_ids=list(range(8)), trace=trace, tmpdir=tmpdir,
    )
    return assemble_output(r.results), r


def kernel(**inputs):
    out, _ = run(inputs, trace=False)
    return out
